# revision 23
# baseline (speedup 1.0000x reference)
"""GPTQ-style 4-bit dequantizer on 8 TRN2 NeuronCores.

Strategy (column-parallel per the N-axis sharding hint):
  - Shard qweight/scales/qzeros/output along N across 8 cores; g_idx replicated.
  - Per core (shard N_S = 1376 columns), PSUM-accumulate pipeline per
    128-row output tile (rows k = 1024t + 8p + j):
      1. PE:  ps  = onehot_u @ s_bf16      (start=True, 3 matmuls of <=512)
      2. DVE: ps  = w * ps                 (in-place f32 tensor_tensor)
      3. PE:  ps += onehot_u @ (-z*s)_bf16 (start=False accumulate)
      4. ACT: ot  = copy(ps)  PSUM->SBUF
      5. DMA strided-row store (rows 8*kpf+j)
    bf16 gathers are single-strip (K=32): |err| <~ 2^-9 * |s|*(w+z) ~ 1e-3,
    well inside the 2e-2 gate.
  - Nibble unpack via one 4x-mode tensor_scalar per shift pair on an int16
    view (nibbles j and j+4 per op).
"""

import numpy as np
from contextlib import ExitStack

import concourse.bacc as bacc
import concourse.bass as bass
import concourse.tile as tile
import concourse.mybir as mybir
from concourse.bass_utils import run_bass_kernel_spmd

K = 4096          # input features (rows of dequantized weight)
N = 11008         # output features
G = 32            # quant groups
PF = 8            # int32 packs 8 nibbles
MAXQ = 0xF
NCORES = 8
NS = N // NCORES        # 1376 columns per core
KP = K // PF            # 512 packed rows
NZS = NS // PF          # 172 packed qzero columns per core
KT = KP // 128          # 4 packed row-tiles
CW = 688                # PSUM chunk width (2 banks); 2 chunks per tile
CHUNKS = [(0, CW), (CW, CW)]
MMSPLIT = [(0, 512), (512, 176)]                # matmul N<=512 within chunk

f32 = mybir.dt.float32
bf16 = mybir.dt.bfloat16
i32 = mybir.dt.int32
i16 = mybir.dt.int16
Alu = mybir.AluOpType

_module_cache = {}


def build_module(n_ktiles=KT):
    nc = bacc.Bacc("TRN2", target_bir_lowering=False, debug=False,
                   num_devices=NCORES)
    qw_d = nc.dram_tensor("qweight", [KP, NS], i32, kind="ExternalInput")
    sc_d = nc.dram_tensor("scales", [G, NS], f32, kind="ExternalInput")
    qz_d = nc.dram_tensor("qzeros", [G, NZS], i32, kind="ExternalInput")
    gi_d = nc.dram_tensor("g_idx", [1, K], i32, kind="ExternalInput")
    out_d = nc.dram_tensor("out", [K, NS], f32, kind="ExternalOutput")

    with tile.TileContext(nc) as tc, ExitStack() as ctx:
        const = ctx.enter_context(tc.tile_pool(name="const", bufs=1))
        qwp = ctx.enter_context(tc.tile_pool(name="qw", bufs=3))
        wfp = ctx.enter_context(tc.tile_pool(name="wf", bufs=4))
        outp = ctx.enter_context(tc.tile_pool(name="out", bufs=4))
        psp = ctx.enter_context(tc.tile_pool(name="ps", bufs=4, space="PSUM"))

        # ---- constants / precompute ----
        # g_idx broadcast first: it heads the longest critical chain
        # (g_b DMA -> is_equal -> first gather matmuls).
        g_b = const.tile([G, K], i32)
        nc.sync.dma_start(g_b[:], bass.AP(gi_d, 0, [[0, G], [1, K]]))
        scales_sb = const.tile([G, NS], f32)
        nc.sync.dma_start(scales_sb[:], sc_d.ap())
        qz_sb = const.tile([G, NZS], i32)
        nc.sync.dma_start(qz_sb[:], qz_d.ap())

        iota_col = const.tile([G, 1], f32)
        nc.gpsimd.iota(iota_col[:], [[0, 1]], channel_multiplier=1,
                       allow_small_or_imprecise_dtypes=True)

        # Zero-pad the contraction dim to K=128: K=32 matmuls leave 3/4 of
        # the PE array rows idle and the HAM clock gate never un-throttles
        # (all matmuls measured at the cold 1.2 GHz rate). Padding costs
        # nothing per-matmul (stream time is N cycles regardless of K).
        # Memsets run on the DVE over i32-bitcast views (half the free-dim
        # elements); GpSimd memsets measured ~3x slower and delay startup.
        s16 = const.tile([128, NS], bf16)
        nc.vector.memset(s16[:].bitcast(i32), 0)
        nc.scalar.copy(s16[0:G, :], scales_sb[:])

        # one-hot in (t, j) block order: block u=t*8+j, col p <-> k = 1024t+8p+j
        # (written as bf16 directly by the DVE is_equal -- no ACT copy)
        onehot = const.tile([128, K], bf16)
        nc.vector.memset(onehot[:].bitcast(i32), 0)
        nz16 = const.tile([128, NS], bf16)
        nc.vector.memset(nz16[:].bitcast(i32), 0)
        g5 = g_b[:].rearrange("p (t q e) -> p t e q", t=KT, q=128, e=PF)
        oh_v = onehot[0:G, :].rearrange("p (t e q) -> p t e q",
                                        t=KT, e=PF, q=128)
        nc.vector.tensor_scalar(oh_v, g5, iota_col[:], None,
                                op0=Alu.is_equal)

        # unpack zeros (int32, strided by 8), then negsz = -(z * s)
        zeros_i = const.tile([G, NS], i32)
        z3 = zeros_i[:].rearrange("p (c e) -> p c e", e=PF)
        for jz in range(PF):
            nc.vector.tensor_scalar(
                z3[:, :, jz], qz_sb[:], 4 * jz, MAXQ,
                op0=Alu.logical_shift_right, op1=Alu.bitwise_and)
        negsz = const.tile([G, NS], f32)
        nc.vector.scalar_tensor_tensor(
            negsz[:], zeros_i[:], -1.0, scales_sb[:],
            op0=Alu.mult, op1=Alu.mult)
        nc.scalar.copy(nz16[0:G, :], negsz[:])

        # PE warm-up: back-to-back matmuls so HAM reaches 8/8 (2.4 GHz)
        # before the gather matmuls start; uses only s16 (ready earliest,
        # no dependence on the one-hot chain).
        warm = psp.tile([128, CW], f32, tag="ps")
        for _ in range(16):
            nc.tensor.matmul(warm[:, 0:512], s16[:, 0:128],
                             s16[:, 0:512], start=True, stop=True)

        # ---- main loop ----
        # Software-pipelined one stage deep: the s-gather matmuls for tile i
        # are emitted on the PE queue BEFORE the nz-accumulate matmuls of
        # tile i-1 (which must wait for DVE), so the PE never head-of-line
        # blocks behind the DVE multiply and stays HAM-warm.
        out4 = out_d.ap().rearrange("(t q e) n -> t q e n",
                                    t=KT, q=128, e=PF)

        def finish(st):
            (p_chunks, p_oh, p_t, p_j) = st
            ot = outp.tile([128, NS], f32)
            for (c0, cw), ps in zip(CHUNKS, p_chunks):
                for (m0, mw) in MMSPLIT:
                    nc.tensor.matmul(ps[:, m0:m0 + mw], p_oh,
                                     nz16[:, c0 + m0:c0 + m0 + mw],
                                     start=False, stop=True,
                                     skip_group_check=True)
                nc.scalar.copy(ot[:, c0:c0 + cw], ps[:])
            nc.sync.dma_start(out4[p_t, :, p_j, :], ot[:])

        pending = None
        for t in range(n_ktiles):
            qw_t = qwp.tile([128, NS], i32)
            nc.sync.dma_start(qw_t[:], qw_d.ap()[t * 128:(t + 1) * 128, :])
            qw16 = qw_t[:].bitcast(i16)
            for tt in range(4):
                wf16 = wfp.tile([128, 2 * NS], i16, tag="wf16")
                nc.vector.tensor_scalar(
                    wf16[:], qw16, 4 * tt, MAXQ,
                    op0=Alu.logical_shift_right, op1=Alu.bitwise_and)
                wf3 = wf16[:].rearrange("p (c e) -> p c e", e=2)
                for l in range(2):
                    j = tt + 4 * l
                    u = t * PF + j
                    oh_u = onehot[:, u * 128:(u + 1) * 128]
                    chunks = []
                    for (c0, cw) in CHUNKS:
                        ps = psp.tile([128, CW], f32, tag="ps")
                        for (m0, mw) in MMSPLIT:
                            nc.tensor.matmul(ps[:, m0:m0 + mw], oh_u,
                                             s16[:, c0 + m0:c0 + m0 + mw],
                                             start=True, stop=True)
                        chunks.append(ps)
                    if pending is not None:
                        finish(pending)
                    for (c0, cw), ps in zip(CHUNKS, chunks):
                        nc.vector.tensor_tensor(
                            ps[:], wf3[:, c0:c0 + cw, l], ps[:], op=Alu.mult)
                    pending = (chunks, oh_u, t, j)
        finish(pending)

    nc.compile()
    return nc


def get_module():
    if "nc" not in _module_cache:
        _module_cache["nc"] = build_module()
    return _module_cache["nc"]


def kernel(qweight, qzeros, scales, g_idx):
    qweight = np.ascontiguousarray(qweight, dtype=np.int32)
    qzeros = np.ascontiguousarray(qzeros, dtype=np.int32)
    scales = np.ascontiguousarray(scales, dtype=np.float32)
    g_idx_2d = np.ascontiguousarray(g_idx, dtype=np.int32).reshape(1, K)

    nc = get_module()
    in_maps = []
    for c in range(NCORES):
        nlo, nhi = c * NS, (c + 1) * NS
        in_maps.append({
            "qweight": np.ascontiguousarray(qweight[:, nlo:nhi]),
            "scales": np.ascontiguousarray(scales[:, nlo:nhi]),
            "qzeros": np.ascontiguousarray(qzeros[:, c * NZS:(c + 1) * NZS]),
            "g_idx": g_idx_2d,
        })
    res = run_bass_kernel_spmd(nc, in_maps, list(range(NCORES))).results
    out = np.concatenate([res[c]["out"] for c in range(NCORES)], axis=1)
    return np.ascontiguousarray(out, dtype=np.float32)


# revision 24
# speedup vs baseline: 1.0746x; 1.0746x over previous
"""GPTQ-style 4-bit dequantizer on 8 TRN2 NeuronCores.

Strategy (column-parallel per the N-axis sharding hint):
  - Shard qweight/scales/qzeros/output along N across 8 cores; g_idx replicated.
  - Per core (shard N_S = 1376 columns), PSUM-accumulate pipeline per
    128-row output tile (rows k = 1024t + 8p + j):
      1. PE:  ps  = onehot_u @ s_bf16      (start=True, 3 matmuls of <=512)
      2. DVE: ps  = w * ps                 (in-place f32 tensor_tensor)
      3. PE:  ps += onehot_u @ (-z*s)_bf16 (start=False accumulate)
      4. ACT: ot  = copy(ps)  PSUM->SBUF
      5. DMA strided-row store (rows 8*kpf+j)
    bf16 gathers are single-strip (K=32): |err| <~ 2^-9 * |s|*(w+z) ~ 1e-3,
    well inside the 2e-2 gate.
  - Nibble unpack via one 4x-mode tensor_scalar per shift pair on an int16
    view (nibbles j and j+4 per op).
"""

import numpy as np
from contextlib import ExitStack

import concourse.bacc as bacc
import concourse.bass as bass
import concourse.tile as tile
import concourse.mybir as mybir
from concourse.bass_utils import run_bass_kernel_spmd

K = 4096          # input features (rows of dequantized weight)
N = 11008         # output features
G = 32            # quant groups
PF = 8            # int32 packs 8 nibbles
MAXQ = 0xF
NCORES = 8
NS = N // NCORES        # 1376 columns per core
KP = K // PF            # 512 packed rows
NZS = NS // PF          # 172 packed qzero columns per core
KT = KP // 128          # 4 packed row-tiles
CW = 688                # PSUM chunk width (2 banks); 2 chunks per tile
CHUNKS = [(0, CW), (CW, CW)]
MMSPLIT = [(0, 512), (512, 176)]                # matmul N<=512 within chunk

f32 = mybir.dt.float32
bf16 = mybir.dt.bfloat16
i32 = mybir.dt.int32
i16 = mybir.dt.int16
Alu = mybir.AluOpType

_module_cache = {}


def build_module(n_ktiles=KT):
    nc = bacc.Bacc("TRN2", target_bir_lowering=False, debug=False,
                   num_devices=NCORES)
    qw_d = nc.dram_tensor("qweight", [KP, NS], i32, kind="ExternalInput")
    sc_d = nc.dram_tensor("scales", [G, NS], f32, kind="ExternalInput")
    qz_d = nc.dram_tensor("qzeros", [G, NZS], i32, kind="ExternalInput")
    gi_d = nc.dram_tensor("g_idx", [1, K], i32, kind="ExternalInput")
    out_d = nc.dram_tensor("out", [K, NS], f32, kind="ExternalOutput")

    with tile.TileContext(nc) as tc, ExitStack() as ctx:
        const = ctx.enter_context(tc.tile_pool(name="const", bufs=1))
        qwp = ctx.enter_context(tc.tile_pool(name="qw", bufs=3))
        wfp = ctx.enter_context(tc.tile_pool(name="wf", bufs=5))
        outp = ctx.enter_context(tc.tile_pool(name="out", bufs=6))
        psp = ctx.enter_context(tc.tile_pool(name="ps", bufs=4, space="PSUM"))

        # ---- constants / precompute ----
        # g_idx broadcast first: it heads the longest critical chain
        # (g_b DMA -> is_equal -> first gather matmuls).
        g_b = const.tile([G, K], i32)
        nc.sync.dma_start(g_b[:], bass.AP(gi_d, 0, [[0, G], [1, K]]))
        scales_sb = const.tile([G, NS], f32)
        nc.sync.dma_start(scales_sb[:], sc_d.ap())
        qz_sb = const.tile([G, NZS], i32)
        nc.sync.dma_start(qz_sb[:], qz_d.ap())

        iota_col = const.tile([G, 1], f32)
        nc.gpsimd.iota(iota_col[:], [[0, 1]], channel_multiplier=1,
                       allow_small_or_imprecise_dtypes=True)

        # Zero-pad the contraction dim to K=128: K=32 matmuls leave 3/4 of
        # the PE array rows idle and the HAM clock gate never un-throttles
        # (all matmuls measured at the cold 1.2 GHz rate). Padding costs
        # nothing per-matmul (stream time is N cycles regardless of K).
        # Memsets run on the DVE over i32-bitcast views (half the free-dim
        # elements); GpSimd memsets measured ~3x slower and delay startup.
        s16 = const.tile([128, NS], bf16)
        nc.vector.memset(s16[:].bitcast(i32), 0)
        nc.scalar.copy(s16[0:G, :], scales_sb[:])

        # one-hot in (t, j) block order: block u=t*8+j, col p <-> k = 1024t+8p+j
        # (written as bf16 directly by the DVE is_equal -- no ACT copy)
        onehot = const.tile([128, K], bf16)
        nc.vector.memset(onehot[:].bitcast(i32), 0)
        nz16 = const.tile([128, NS], bf16)
        nc.vector.memset(nz16[:].bitcast(i32), 0)
        g5 = g_b[:].rearrange("p (t q e) -> p t e q", t=KT, q=128, e=PF)
        oh_v = onehot[0:G, :].rearrange("p (t e q) -> p t e q",
                                        t=KT, e=PF, q=128)
        nc.vector.tensor_scalar(oh_v, g5, iota_col[:], None,
                                op0=Alu.is_equal)

        # unpack zeros (int32, strided by 8), then negsz = -(z * s)
        zeros_i = const.tile([G, NS], i32)
        z3 = zeros_i[:].rearrange("p (c e) -> p c e", e=PF)
        for jz in range(PF):
            nc.vector.tensor_scalar(
                z3[:, :, jz], qz_sb[:], 4 * jz, MAXQ,
                op0=Alu.logical_shift_right, op1=Alu.bitwise_and)
        negsz = const.tile([G, NS], f32)
        nc.vector.scalar_tensor_tensor(
            negsz[:], zeros_i[:], -1.0, scales_sb[:],
            op0=Alu.mult, op1=Alu.mult)
        nc.scalar.copy(nz16[0:G, :], negsz[:])

        # PE warm-up: back-to-back matmuls so HAM reaches 8/8 (2.4 GHz)
        # before the gather matmuls start; uses only s16 (ready earliest,
        # no dependence on the one-hot chain).
        warm = psp.tile([128, CW], f32, tag="ps")
        for _ in range(16):
            nc.tensor.matmul(warm[:, 0:512], s16[:, 0:128],
                             s16[:, 0:512], start=True, stop=True)

        # ---- main loop ----
        # Software-pipelined one stage deep: the s-gather matmuls for tile i
        # are emitted on the PE queue BEFORE the nz-accumulate matmuls of
        # tile i-1 (which must wait for DVE), so the PE never head-of-line
        # blocks behind the DVE multiply and stays HAM-warm.
        out4 = out_d.ap().rearrange("(t q e) n -> t q e n",
                                    t=KT, q=128, e=PF)

        def finish(st):
            (p_chunks, p_oh, p_t, p_j) = st
            ot = outp.tile([128, NS], f32)
            for (c0, cw), ps in zip(CHUNKS, p_chunks):
                for (m0, mw) in MMSPLIT:
                    nc.tensor.matmul(ps[:, m0:m0 + mw], p_oh,
                                     nz16[:, c0 + m0:c0 + m0 + mw],
                                     start=False, stop=True,
                                     skip_group_check=True)
                nc.scalar.copy(ot[:, c0:c0 + cw], ps[:])
            nc.sync.dma_start(out4[p_t, :, p_j, :], ot[:])

        pending = None
        for t in range(n_ktiles):
            qw_t = qwp.tile([128, NS], i32)
            nc.sync.dma_start(qw_t[:], qw_d.ap()[t * 128:(t + 1) * 128, :])
            qw16 = qw_t[:].bitcast(i16)
            for tt in range(4):
                wf16 = wfp.tile([128, 2 * NS], i16, tag="wf16")
                nc.vector.tensor_scalar(
                    wf16[:], qw16, 4 * tt, MAXQ,
                    op0=Alu.logical_shift_right, op1=Alu.bitwise_and)
                wf3 = wf16[:].rearrange("p (c e) -> p c e", e=2)
                for l in range(2):
                    j = tt + 4 * l
                    u = t * PF + j
                    oh_u = onehot[:, u * 128:(u + 1) * 128]
                    chunks = []
                    for (c0, cw) in CHUNKS:
                        ps = psp.tile([128, CW], f32, tag="ps")
                        for (m0, mw) in MMSPLIT:
                            nc.tensor.matmul(ps[:, m0:m0 + mw], oh_u,
                                             s16[:, c0 + m0:c0 + m0 + mw],
                                             start=True, stop=True)
                        chunks.append(ps)
                    if pending is not None:
                        finish(pending)
                    for (c0, cw), ps in zip(CHUNKS, chunks):
                        nc.vector.tensor_tensor(
                            ps[:], wf3[:, c0:c0 + cw, l], ps[:], op=Alu.mult)
                    pending = (chunks, oh_u, t, j)
        finish(pending)

    nc.compile()
    return nc


def get_module():
    if "nc" not in _module_cache:
        _module_cache["nc"] = build_module()
    return _module_cache["nc"]


def kernel(qweight, qzeros, scales, g_idx):
    qweight = np.ascontiguousarray(qweight, dtype=np.int32)
    qzeros = np.ascontiguousarray(qzeros, dtype=np.int32)
    scales = np.ascontiguousarray(scales, dtype=np.float32)
    g_idx_2d = np.ascontiguousarray(g_idx, dtype=np.int32).reshape(1, K)

    nc = get_module()
    in_maps = []
    for c in range(NCORES):
        nlo, nhi = c * NS, (c + 1) * NS
        in_maps.append({
            "qweight": np.ascontiguousarray(qweight[:, nlo:nhi]),
            "scales": np.ascontiguousarray(scales[:, nlo:nhi]),
            "qzeros": np.ascontiguousarray(qzeros[:, c * NZS:(c + 1) * NZS]),
            "g_idx": g_idx_2d,
        })
    res = run_bass_kernel_spmd(nc, in_maps, list(range(NCORES))).results
    out = np.concatenate([res[c]["out"] for c in range(NCORES)], axis=1)
    return np.ascontiguousarray(out, dtype=np.float32)
